# revision 12
# baseline (speedup 1.0000x reference)
"""Causal attention (RoPE, 16 heads, L=2048, H=2048) on 8 trn2 NeuronCores.

Sharding: tensor-parallel over heads. Core i handles heads 2i, 2i+1
(d=128 each): column-parallel q/k/v projections, row-parallel o_proj,
host-side sum of the 8 partial outputs.

Per-core device kernel (all matmuls fp32r = full-rate PE):
  - Q^T, K^T computed in [d, L] layout (weight-stationary matmuls, N=512),
    RoPE applied during PSUM->SBUF eviction on DVE.
  - V computed in natural [L, d] layout (x-stationary matmuls, N=256).
  - Attention per (head, q-chunk of 512): S^T = K Q^T on PE; exp on ACT
    (softmax scale pre-folded into Wq); causal mask via gpsimd affine_select
    on diagonal tiles only (fully-masked tiles never computed); softmax
    denominators via an all-ones matmul (replicated to 128 partitions);
    unnormalized O^T accumulated over k-tiles; normalized by DVE
    reciprocal+mul into [d, L] layout.
  - o_proj: out_partial[q, H] from lhsT=O^T chunks, rhs=Wo slice.
"""
import numpy as np

L = 2048
H = 2048
NH = 16
D = 128          # head dim
NCORES = 8
HPC = NH // NCORES   # heads per core = 2
ROPE_BASE = 10000.0

_CACHE = {}


def _rope_tables():
    inv_freq = 1.0 / (ROPE_BASE ** (np.arange(0, D, 2, dtype=np.float32) / D))
    t = np.arange(L, dtype=np.float32)
    freqs = np.outer(t, inv_freq).astype(np.float32)          # [L, D/2]
    emb = np.concatenate([freqs, freqs], axis=-1)             # [L, D]
    cos = np.cos(emb).astype(np.float32)                      # [L, D]
    sin = np.sin(emb).astype(np.float32)
    cosT = np.ascontiguousarray(cos.T)                        # [D, L]
    sinT = np.ascontiguousarray(sin.T)
    sinTs = sinT.copy()
    sinTs[: D // 2] = -sinT[: D // 2]                         # sign-folded
    return cosT, sinTs


def _build_nc():
    import concourse.bacc as bacc
    import concourse.mybir as mybir
    from concourse import tile
    from contextlib import ExitStack

    f32 = mybir.dt.float32
    f32r = mybir.dt.float32r
    AF = mybir.ActivationFunctionType
    OP = mybir.AluOpType

    nc = bacc.Bacc("TRN2", target_bir_lowering=False, debug=False)

    xT_d = nc.dram_tensor("xT", (H, L), f32r, kind="ExternalInput")
    wq_d = nc.dram_tensor("wqT", (H, HPC * D), f32r, kind="ExternalInput")
    wk_d = nc.dram_tensor("wkT", (H, HPC * D), f32r, kind="ExternalInput")
    wv_d = nc.dram_tensor("wvT", (H, HPC * D), f32r, kind="ExternalInput")
    wo_d = nc.dram_tensor("woP", (HPC * D, H), f32r, kind="ExternalInput")
    cos_d = nc.dram_tensor("cosT", (D, L), f32, kind="ExternalInput")
    sin_d = nc.dram_tensor("sinTs", (D, L), f32, kind="ExternalInput")
    id_d = nc.dram_tensor("ident", (128, 128), f32r, kind="ExternalInput")
    out_d = nc.dram_tensor("out", (L, H), f32, kind="ExternalOutput")

    KC = H // 128        # 16 contraction chunks
    LCN = 4              # L chunks of 512 in projections
    QCN = 4              # q chunks of 512 in attention
    LT = L // 128        # 16 L tiles

    with tile.TileContext(nc) as tc, ExitStack() as top:
        per = top.enter_context(tc.tile_pool(name="per", bufs=1))

        # persistent tiles; q/k/o split per (head, L-chunk) so later phases
        # can start as soon as their chunk is ready
        wo_sb = per.tile([128, HPC, H], f32r)
        cos_sb = per.tile([128, L], f32)
        sin_sb = per.tile([128, L], f32)
        qt_sb = [[per.tile([128, 512], f32r, name=f"qt{h}_{c}")
                  for c in range(LCN)] for h in range(HPC)]
        kt_sb = [[per.tile([128, 512], f32r, name=f"kt{h}_{c}")
                  for c in range(LCN)] for h in range(HPC)]
        v_sb = [[per.tile([128, 4, D], f32r, name=f"v{h}_{c}")
                 for c in range(LCN)] for h in range(HPC)]
        ot_sb = [[per.tile([128, 512], f32r, name=f"ot{h}_{c}")
                  for c in range(QCN)] for h in range(HPC)]
        ones_f = per.tile([128, 128], f32)
        ones_r = per.tile([128, 128], f32r)
        ident = per.tile([128, 128], f32r)
        # causal masks for the 4 diagonal-block variants (j = kt - 4*qc):
        # keep iff y - x - 128*j >= 0
        masks = [per.tile([128, 512], f32r, name=f"mask{j}") for j in range(4)]

        nc.vector.memset(ones_f[:], 1.0)
        nc.vector.tensor_copy(ones_r[:], ones_f[:])
        nc.gpsimd.dma_start(ident[:], id_d[:])
        mstage = per.tile([128, 512], f32)
        for j in range(4):
            nc.gpsimd.memset(mstage[:], 1.0)
            nc.gpsimd.affine_select(
                mstage[:], mstage[:], pattern=[[1, 512]],
                compare_op=OP.is_ge, fill=0.0, base=-128 * j,
                channel_multiplier=-1)
            nc.vector.tensor_copy(masks[j][:], mstage[:])

        # ---------------- projections ----------------
        with ExitStack() as proj:
            wpool = proj.enter_context(tc.tile_pool(name="w", bufs=1))
            xpool = proj.enter_context(tc.tile_pool(name="x", bufs=8))
            tpool = proj.enter_context(tc.tile_pool(name="ropetmp", bufs=4))
            vtpool = proj.enter_context(tc.tile_pool(name="vt", bufs=3))
            qpps = proj.enter_context(tc.tile_pool(name="qpps", bufs=2, space="PSUM"))
            pps = proj.enter_context(tc.tile_pool(name="pps", bufs=1, space="PSUM"))

            wq_sb = wpool.tile([128, KC, HPC * D], f32r)
            wk_sb = wpool.tile([128, KC, HPC * D], f32r)
            wv_sb = wpool.tile([128, KC, HPC * D], f32r)
            wqr = wq_d.rearrange("(c p) n -> c p n", p=128)
            wkr = wk_d.rearrange("(c p) n -> c p n", p=128)
            wvr = wv_d.rearrange("(c p) n -> c p n", p=128)

            for lc in range(LCN):
                qps = [qpps.tile([128, 512], f32, tag=f"qps{h}", name=f"qps{h}") for h in range(HPC)]
                kps = [pps.tile([128, 512], f32, tag=f"kps{h}", name=f"kps{h}") for h in range(HPC)]
                vps = [pps.tile([128, 512], f32, tag=f"vps{i}", name=f"vps{i}") for i in range(HPC)]
                for kc in range(KC):
                    xt = xpool.tile([128, 512], f32r, tag="xt")
                    nc.sync.dma_start(
                        xt[:], xT_d[kc * 128:(kc + 1) * 128, lc * 512:(lc + 1) * 512])
                    if lc == 0:
                        # stream weights chunk-by-chunk so the first matmuls
                        # start after ~384KB of DMA, not 8MB
                        nc.scalar.dma_start(wq_sb[:, kc, :], wqr[kc])
                        nc.scalar.dma_start(wk_sb[:, kc, :], wkr[kc])
                        nc.scalar.dma_start(wv_sb[:, kc, :], wvr[kc])
                        if kc == 0:
                            nc.gpsimd.dma_start(cos_sb[:], cos_d[:])
                            nc.gpsimd.dma_start(sin_sb[:], sin_d[:])
                    st, sp = (kc == 0), (kc == KC - 1)
                    for h in range(HPC):
                        nc.tensor.matmul(qps[h][:], wq_sb[:, kc, h * D:(h + 1) * D],
                                         xt[:], start=st, stop=sp)
                        nc.tensor.matmul(kps[h][:], wk_sb[:, kc, h * D:(h + 1) * D],
                                         xt[:], start=st, stop=sp)
                        nc.tensor.matmul(vps[h][:], wv_sb[:, kc, h * D:(h + 1) * D],
                                         xt[:], start=st, stop=sp)
                # RoPE: dst = src*cos + swap(src)*sin_signed  (DVE, f32r out)
                cs = slice(lc * 512, (lc + 1) * 512)
                for h in range(HPC):
                    for src, dst in ((qps[h], qt_sb[h][lc]), (kps[h], kt_sb[h][lc])):
                        t1 = tpool.tile([128, 512], f32, tag="t1")
                        t2 = tpool.tile([128, 512], f32, tag="t2")
                        nc.vector.tensor_tensor(
                            t1[0:64, :], src[64:128, :], sin_sb[0:64, cs], OP.mult)
                        nc.vector.tensor_tensor(
                            t1[64:128, :], src[0:64, :], sin_sb[64:128, cs], OP.mult)
                        nc.vector.tensor_tensor(t2[:], src[:], cos_sb[:, cs], OP.mult)
                        nc.vector.tensor_tensor(dst[:], t1[:], t2[:], OP.add)
                # V^T -> SBUF, then PE-transpose each 128x128 into natural V
                for h in range(HPC):
                    vt = vtpool.tile([128, 512], f32r, tag="vt")
                    nc.scalar.copy(vt[:], vps[h][:])
                    for lt in range(4):
                        tp = pps.tile([128, 128], f32r, tag=f"vps{h}", name="tp")
                        nc.tensor.transpose(tp[:], vt[:, lt * 128:(lt + 1) * 128],
                                            ident[:])
                        nc.scalar.copy(v_sb[h][lc][:, lt, :], tp[:])

        # -------- attention (qc-major) interleaved with o_proj --------
        with ExitStack() as att:
            ppool = att.enter_context(tc.tile_pool(name="pt", bufs=8))
            rpool = att.enter_context(tc.tile_pool(name="recip", bufs=2))
            obuf = att.enter_context(tc.tile_pool(name="ob", bufs=6))
            sps_p = att.enter_context(tc.tile_pool(name="sps", bufs=2, space="PSUM"))
            acc_p = att.enter_context(tc.tile_pool(name="acc", bufs=2, space="PSUM"))
            ops_p = att.enter_context(tc.tile_pool(name="ops", bufs=2, space="PSUM"))

            nc.scalar.dma_start(wo_sb[:], wo_d.rearrange("(c p) n -> p c n", p=128))

            for qc in range(QCN):
                n_kt = 4 * qc + 4
                for h in range(HPC):
                    sums = acc_p.tile([128, 512], f32, tag="sums")
                    ops = acc_p.tile([128, 512], f32, tag="ops")
                    for kt in range(n_kt):
                        st, sp = (kt == 0), (kt == n_kt - 1)
                        s_ps = sps_p.tile([128, 512], f32, tag="s")
                        nc.tensor.matmul(
                            s_ps[:], kt_sb[h][kt // 4][:, (kt % 4) * 128:(kt % 4 + 1) * 128],
                            qt_sb[h][qc][:], start=True, stop=True)
                        pt = ppool.tile([128, 512], f32r, tag="pt")
                        nc.scalar.activation(pt[:], s_ps[:], AF.Exp)
                        if kt >= 4 * qc:
                            nc.gpsimd.affine_select(
                                pt[:], pt[:], pattern=[[1, 512]],
                                compare_op=OP.is_ge, fill=0.0,
                                base=512 * qc - 128 * kt, channel_multiplier=-1)
                        nc.tensor.matmul(sums[:], ones_r[:], pt[:], start=st, stop=sp)
                        nc.tensor.matmul(ops[:], v_sb[h][kt // 4][:, kt % 4, :], pt[:],
                                         start=st, stop=sp)
                    recip = rpool.tile([128, 512], f32, tag="recip")
                    nc.vector.reciprocal_approx_fast(recip[:], sums[:])
                    nc.vector.tensor_tensor(ot_sb[h][qc][:], ops[:], recip[:], OP.mult)
                # o_proj for the q-tiles of this q-chunk
                for qt4 in range(4):
                    for hcn in range(4):
                        po = ops_p.tile([128, 512], f32, tag="po")
                        for h in range(HPC):
                            nc.tensor.matmul(
                                po[:], ot_sb[h][qc][:, qt4 * 128:(qt4 + 1) * 128],
                                wo_sb[:, h, hcn * 512:(hcn + 1) * 512],
                                start=(h == 0), stop=(h == HPC - 1))
                        ob = obuf.tile([128, 512], f32, tag="ob")
                        if hcn % 2 == 0:
                            nc.vector.tensor_copy(ob[:], po[:])
                        else:
                            nc.scalar.copy(ob[:], po[:])
                        qt = qc * 4 + qt4
                        nc.sync.dma_start(
                            out_d[qt * 128:(qt + 1) * 128, hcn * 512:(hcn + 1) * 512],
                            ob[:])

    nc.compile()
    return nc


def _prep_inputs(x, Wq, Wk, Wv, Wo):
    xT = np.ascontiguousarray(x.reshape(L, H).T).astype(np.float32)
    cosT, sinTs = _rope_tables()
    ident = np.eye(128, dtype=np.float32)
    scale = np.float32(1.0 / np.sqrt(D))
    in_maps = []
    for i in range(NCORES):
        rs = slice(i * HPC * D, (i + 1) * HPC * D)
        in_maps.append({
            "xT": xT,
            "wqT": np.ascontiguousarray(Wq[rs].T * scale),
            "wkT": np.ascontiguousarray(Wk[rs].T),
            "wvT": np.ascontiguousarray(Wv[rs].T),
            "woP": np.ascontiguousarray(Wo[:, rs].T),
            "cosT": cosT,
            "sinTs": sinTs,
            "ident": ident,
        })
    return in_maps


def run(x, Wq, Wk, Wv, Wo, trace=False):
    from concourse.bass_utils import run_bass_kernel_spmd
    if "nc" not in _CACHE:
        _CACHE["nc"] = _build_nc()
    nc = _CACHE["nc"]
    in_maps = _prep_inputs(np.asarray(x), np.asarray(Wq), np.asarray(Wk),
                           np.asarray(Wv), np.asarray(Wo))
    res = run_bass_kernel_spmd(nc, in_maps, core_ids=list(range(NCORES)),
                               trace=trace)
    acc = np.zeros((L, H), dtype=np.float64)
    for r in res.results:
        acc += r["out"].astype(np.float64)
    return acc.astype(np.float32).reshape(1, L, H), res


def kernel(x, Wq, Wk, Wv, Wo):
    out, _ = run(x, Wq, Wk, Wv, Wo)
    return out
